# revision 4
# baseline (speedup 1.0000x reference)
"""ChamferLoss Trainium2 kernel v5 — engine-parallel extraction pipeline.

Data-parallel over batch: 16 batches / 8 cores = 2 each.
  m[b,i,j] = -pdist = 2 x_i.y_j - ||x_i||^2 - ||y_j||^2   (first 3 channels)
  loss = -( mean_bi max_j m + mean_bj max_i m )

Cross term: K=13 bf16 augmented matmul (hi/lo split), operands built on host
as [13, b_loc, 2, n] bf16 (side 0 = x weights, side 1 = y moving), DMA'd
per-batch so batch 0 compute starts before batch 1 arrives.

Per (batch b, row-tile quad q): PE fills 8 [128,2048] PSUM chunks (4x512-wide
MMs each, bufs=2 ping-pong). ACT (ScalarE) extracts each chunk to bf16 SBUF
(cp2 [128,4,4096]). DVE consumes cp2 at 2x (bf16 packed):
  cols: colacc_b = max(colacc_b, cp2[:,t,:]) chained TT (tile 0: plain copy,
        4x mode, replaces the memset)
  rows: paired TT-max tree over both tiles' j-halves (3D APs), one 1x
        tensor_reduce -> rowpart[:, 2 cols]
Col finals per batch (overlapped with the next batch's main loop): PE
transposes 128x128 colacc blocks into PSUM, DVE max-reduces the transposed
partition axis into cmax. Tail: add-reduces + one DMA of partial [128,4].
Host sums partial over cores/partitions.
"""

from contextlib import ExitStack

import numpy as np

import concourse.bass as bass
import concourse.bacc as bacc
import concourse.tile as tile
from concourse import bass_isa, mybir
from concourse.bass import ds
from concourse.bass_utils import run_bass_kernel_spmd

F32 = mybir.dt.float32
BF16 = mybir.dt.bfloat16
AX = mybir.AxisListType
OP = mybir.AluOpType

NEG_BIG = -3.0e38

B_FULL = 16
N_FULL = 4096
C_FULL = 6
N_CORES = 8
KAUG = 13


def build_nc(b_loc=2, n=4096, num_devices=8, reps=1, skip_rows=False,
             skip_cols=False, skip_act=False, col_finals="pe"):
    NP = 128
    NQ = n // NP                 # row-tiles per batch (32)
    CW = n // 2                  # psum chunk width (2048)

    nc = bacc.Bacc("TRN2", target_bir_lowering=False, debug=False,
                   enable_asserts=False, num_devices=num_devices)

    aug_d = nc.declare_dram_parameter("aug", [KAUG, b_loc, 2, n], BF16,
                                      isOutput=False).ap()
    iden_d = nc.declare_dram_parameter("iden", [NP, NP], BF16,
                                       isOutput=False).ap()
    out_d = nc.declare_dram_parameter("partial", [NP, 2 * b_loc], F32,
                                      isOutput=True).ap()

    with tile.TileContext(nc) as tc, ExitStack() as ctx:
        singles = ctx.enter_context(tc.tile_pool(name="singles", bufs=1))
        cp_pool = ctx.enter_context(tc.tile_pool(name="cp", bufs=3))
        psum_pool = ctx.enter_context(
            tc.tile_pool(name="psum", bufs=2, space="PSUM"))

        aug_s = singles.tile([KAUG, b_loc, 2, n], BF16, tag="aug",
                             name="aug_s")
        iden = singles.tile([NP, NP], BF16, tag="iden", name="iden")
        colacc = [singles.tile([NP, n], BF16, tag=f"colacc{b}",
                               name=f"colacc{b}") for b in range(b_loc)]
        rowpart = singles.tile([NP, b_loc * NQ], F32, tag="rowpart",
                               name="rowpart")
        t1q = singles.tile([NP, 4, CW], BF16, tag="t1", name="t1q")
        t2q = singles.tile([NP, 4, CW // 2], BF16, tag="t2", name="t2q")
        t3q = singles.tile([NP, 4, CW // 4], BF16, tag="t3", name="t3q")
        t4a = singles.tile([NP, 2, 4, CW // 8], BF16, tag="t4", name="t4a")
        sums = singles.tile([NP, 2 * b_loc], F32, tag="sums", name="sums")
        cmax = singles.tile([NP, b_loc * NQ], F32, tag="cmax", name="cmax")
        cm = (singles.tile([NP, b_loc, n], BF16, tag="cm", name="cm")
              if col_finals == "gps" else None)

        def emit_batch(b, after_first_group=None):
            xw = aug_s[:, b, 0, :]           # [13, n] x-side (weights)
            yv = aug_s[:, b, 1, :]           # [13, n] y-side (moving)
            for q in range(NQ // 4):
                cp2 = cp_pool.tile([NP, 4, n], BF16, tag="cp",
                                   name=f"cp_{b}_{q}")
                for t in range(4):
                    r = 4 * q + t
                    for ch in range(2):
                        ps = psum_pool.tile([NP, CW], F32, tag="ps",
                                            name=f"ps_{b}_{r}_{ch}")
                        for s in range(CW // 512):
                            j0 = ch * CW + s * 512
                            nc.tensor.matmul(
                                ps[:, s * 512:(s + 1) * 512],
                                lhsT=xw[:, r * NP:(r + 1) * NP],
                                rhs=yv[:, j0:j0 + 512],
                                start=True, stop=True)
                        if not skip_act:
                            nc.scalar.copy(
                                cp2[:, t, ch * CW:(ch + 1) * CW], ps)
                        else:
                            nc.vector.tensor_reduce(
                                rowpart[:, ds(b * NQ + r, 1)], ps[:, 0:512],
                                axis=AX.X, op=OP.max)
                    if skip_act or skip_cols:
                        continue
                    # cols: chained elementwise max into colacc
                    if r == 0:
                        nc.vector.tensor_copy(colacc[b], cp2[:, 0, :])
                    else:
                        nc.vector.tensor_tensor(colacc[b], colacc[b],
                                                cp2[:, t, :], op=OP.max)
                if not (skip_act or skip_rows):
                    # rows: quad bf16 TT-max tree; the 1x reduce tail is
                    # amortized over 2 groups via the stacked t4 buffer
                    nc.vector.tensor_tensor(t1q, cp2[:, :, 0:CW],
                                            cp2[:, :, CW:n], op=OP.max)
                    nc.vector.tensor_tensor(t2q, t1q[:, :, 0:CW // 2],
                                            t1q[:, :, CW // 2:CW], op=OP.max)
                    nc.vector.tensor_tensor(t3q, t2q[:, :, 0:CW // 4],
                                            t2q[:, :, CW // 4:CW // 2],
                                            op=OP.max)
                    nc.vector.tensor_tensor(t4a[:, q % 2, :, :],
                                            t3q[:, :, 0:CW // 8],
                                            t3q[:, :, CW // 8:CW // 4],
                                            op=OP.max)
                    if q % 2 == 1:
                        nc.vector.tensor_reduce(
                            rowpart[:, ds(b * NQ + 8 * (q // 2), 8)],
                            t4a, axis=AX.X, op=OP.max)
                if q == 0 and after_first_group is not None:
                    after_first_group()

        def emit_col_finals(b):
            if skip_act or skip_cols:
                return
            if col_finals == "gps":
                nc.gpsimd.partition_all_reduce(
                    cm[:, b, :], colacc[b], NP, bass_isa.ReduceOp.max)
                return
            for g in range(2):
                pst = psum_pool.tile([NP, 2048], BF16, tag="ps",
                                     name=f"pst_{b}_{g}")
                for blk in range(16):
                    j0 = (g * 16 + blk) * NP
                    nc.tensor.transpose(pst[:, blk * NP:(blk + 1) * NP],
                                        colacc[b][:, j0:j0 + NP], iden)
                nc.vector.tensor_reduce(
                    cmax[:, ds(b * NQ + g * 16, 16)],
                    pst.rearrange("p (k i) -> p k i", k=16),
                    axis=AX.X, op=OP.max)

        def emit_body():
            nc.sync.dma_start(out=iden, in_=iden_d)
            for b in range(b_loc):
                nc.sync.dma_start(out=aug_s[:, b, 0, :], in_=aug_d[:, b, 0, :])
                nc.sync.dma_start(out=aug_s[:, b, 1, 0:CW],
                                  in_=aug_d[:, b, 1, 0:CW])
                nc.sync.dma_start(out=aug_s[:, b, 1, CW:n],
                                  in_=aug_d[:, b, 1, CW:n])
            if skip_rows or skip_cols or skip_act:
                nc.vector.memset(rowpart, 0.0)
                nc.vector.memset(cmax, 0.0)
            for b in range(b_loc):
                hook = (lambda bb=b - 1: emit_col_finals(bb)) if b > 0 else None
                emit_batch(b, after_first_group=hook)
            emit_col_finals(b_loc - 1)

            # row finals: sum rowmax over tiles, per partition
            nc.vector.tensor_reduce(
                sums[:, 0:b_loc],
                rowpart.rearrange("p (b q) -> p b q", b=b_loc),
                axis=AX.X, op=OP.add)
            if col_finals == "gps":
                # every partition of cm holds the same per-j col max; scale
                # by 1/128 so the host-side sum over partitions is correct.
                nc.vector.tensor_reduce(sums[:, b_loc:2 * b_loc], cm,
                                        axis=AX.X, op=OP.add)
                nc.scalar.mul(sums[:, b_loc:2 * b_loc],
                              sums[:, b_loc:2 * b_loc], 1.0 / NP)
            else:
                nc.vector.tensor_reduce(
                    sums[:, b_loc:2 * b_loc],
                    cmax.rearrange("p (b q) -> p b q", b=b_loc),
                    axis=AX.X, op=OP.add)
            nc.sync.dma_start(out=out_d, in_=sums)

        if reps == 1:
            emit_body()
        else:
            with tc.For_i(0, reps, 1) as _rep:
                emit_body()

    nc.compile()
    return nc


def _host_aug(x: np.ndarray, y: np.ndarray) -> np.ndarray:
    """Augmented operands for one core: [13, b, 2, n] bf16 hi/lo.
    x, y: [b, n, 6] f32; coordinate channels are the first 3."""
    import ml_dtypes

    b, n, _ = x.shape
    xc = np.ascontiguousarray(x[:, :, :3]).astype(np.float32)
    yc = np.ascontiguousarray(y[:, :, :3]).astype(np.float32)

    def split(v):
        hi = v.astype(ml_dtypes.bfloat16).astype(np.float32)
        lo = (v - hi).astype(ml_dtypes.bfloat16).astype(np.float32)
        return hi, lo

    xh, xl = split(xc)
    rx = np.sum(xc * xc, axis=-1)
    rxh, rxl = split(rx)
    Y = 2.0 * yc
    Yh, Yl = split(Y)
    ry = np.sum(yc * yc, axis=-1)
    ryh, ryl = split(ry)
    ones = np.ones_like(rx)

    # x-side rows (K=13): [xh(3), xh(3), xl(3), -rxh, -rxl, -1, -1]
    ax = np.concatenate(
        [xh, xh, xl, -rxh[..., None], -rxl[..., None],
         -ones[..., None], -ones[..., None]], axis=-1)
    # y-side rows: [Yh(3), Yl(3), Yh(3), 1, 1, ryh, ryl]
    ay = np.concatenate(
        [Yh, Yl, Yh, ones[..., None], ones[..., None],
         ryh[..., None], ryl[..., None]], axis=-1)

    aug = np.empty((KAUG, b, 2, n), dtype=ml_dtypes.bfloat16)
    for bi in range(b):
        aug[:, bi, 0, :] = ax[bi].T.astype(ml_dtypes.bfloat16)
        aug[:, bi, 1, :] = ay[bi].T.astype(ml_dtypes.bfloat16)
    return aug


def _host_iden() -> np.ndarray:
    import ml_dtypes
    return np.eye(128, dtype=ml_dtypes.bfloat16)


_CACHE = {}


def _get_nc():
    if "nc" not in _CACHE:
        _CACHE["nc"] = build_nc(b_loc=B_FULL // N_CORES, n=N_FULL,
                                num_devices=N_CORES)
    return _CACHE["nc"]


def make_in_maps(x: np.ndarray, y: np.ndarray):
    bl = B_FULL // N_CORES
    iden = _host_iden()
    return [
        {"aug": _host_aug(x[i * bl:(i + 1) * bl], y[i * bl:(i + 1) * bl]),
         "iden": iden}
        for i in range(N_CORES)
    ]


def gather_loss(res) -> float:
    total = 0.0
    for r in res.results:
        total += float(r["partial"].astype(np.float64).sum())
    return -total / float(B_FULL * N_FULL)


def kernel(x: np.ndarray, y: np.ndarray) -> np.ndarray:
    x = np.ascontiguousarray(np.asarray(x, dtype=np.float32))
    y = np.ascontiguousarray(np.asarray(y, dtype=np.float32))
    assert x.shape == (B_FULL, N_FULL, C_FULL), x.shape
    nc = _get_nc()
    in_maps = make_in_maps(x, y)
    res = run_bass_kernel_spmd(nc, in_maps, list(range(N_CORES)))
    return np.float32(gather_loss(res))
